# revision 45
# baseline (speedup 1.0000x reference)
"""Trainium2 Bass kernel for nn_AttentionWithTime (B=4, N=2048, in_c=512,
head_c=64, H=8, expand_c=2048, time_c=256), 8-core SPMD.

Sharding: token-parallel. Core c handles batch b=c//2 and query rows
(c%2)*1024 .. +1024 of that batch. Each core computes K/V for its whole
batch (2x redundant) so no cross-core collectives are needed; the host
splits inputs and concatenates the 8 per-core [1024, 512] outputs.

Host-side folds (pure input math, done once in kernel()):
  - ln1 gamma/beta folded into qkv_w/qkv_b
  - attention scale (head_c^-0.5) folded into the q columns
  - v bias folded into merge_b (softmax rows sum to 1)
  - merge_w folded into wv per head: WM_h = wv_h @ merge_w_h * S_SCALE
    (fp8; the scale keeps the product in e4m3's normal range and rides
    on the softmax reciprocal). This removes the on-device merge matmul:
    out_h = (A_h @ Z_h) * recT with Z = ln1(x) @ WM.
  - ln2 gamma/beta folded into ff1_w/ff1_b

Device pipeline (per core):
  A: LN1 + bf16 transposes -> XT (bf16) + one wide fp8 cast -> XT8;
     q^T/k^T. (Z production lives entirely in the units: unit u makes
     head u's Z, one chunk per key slot.)
  C: 8 software-pipelined units (u = head-pair*2 + query-tile). Unit u's
     kc loop interleaves, per key chunk: the 2-head row-packed S matmuls
     (K=64 at row groups 0-63/64-127, concurrent in the PE, exp per kc
     from double-buffered PSUM), the A@Z matmuls of unit u-2
     (A-stationary fp8 DoubleRow, token-major out, as two 257/256-col
     accumulation chains; Z carries a constant S_SCALE column at 256 so
     chain A's column 256 IS the softmax denominator — no separate
     reduction), and Z production per the ZPROD schedule (front-loaded
     into the ACT-bound units 0/1). Each A@Z group evicts via one DVE
     reciprocal + scalar_tensor_tensor into X2.
  D: LN2 (time concat) + bf16 transposes + FFN, overlapping the flush
     of units 6/7. gpsimd only ever runs DMA issues + partition
     broadcasts (mixing op families forces ~7us microcode reloads).
"""

import numpy as np

B, N, IN_C, HEAD_C, EXPAND_C, OUT_C, TIME_C, H = 4, 2048, 512, 64, 2048, 512, 256, 8
EPS = 1e-5
NCORES = 8
NTOK = N  # tokens per batch handled per core (keys)
NQ = N // 2  # query rows owned per core
P = 128
S_SCALE = 16.0  # host scale on WM = wv@merge_w before fp8 cast
F1_SCALE = 256.0  # host scale on ff1_w before fp8 cast (undone in gelu scale)
F2_SCALE = 256.0  # host scale on ff2_w before fp8 cast (undone at eviction)

QUANT = "bf16"


def _emit(nc, tc, tens, dt_op):
    import concourse.bass as bass
    from concourse import mybir
    from concourse.masks import make_identity
    from concourse.bass import ts

    f32 = mybir.dt.float32
    f32r = mybir.dt.float32r
    dt8 = mybir.dt.float8e4
    AF = mybir.ActivationFunctionType
    ALU = mybir.AluOpType

    x_roll = tens["x_roll"][:]
    wqk_d = tens["wqk"][:]
    bqk_d = tens["bqk"][:]
    wm_d = tens["wm"][:]
    mb_d = tens["merge_b"][:]
    t_d = tens["t_vec"][:]
    tw_d = tens["time_w"][:]
    tb_d = tens["time_b"][:]
    f1w_d = tens["ff1_w"][:]
    f1b_d = tens["ff1_b"][:]
    f2w_d = tens["ff2_w"][:]
    f2b_d = tens["ff2_b"][:]
    out_d = tens["out"][:]

    KC = IN_C // P  # 4 feature chunks of x
    TCH = NTOK // P  # 16 token chunks per batch
    QCH = NQ // P  # 8 own-token chunks
    NQT = NQ // 512  # 2 query tiles of 512
    NU = H  # pipeline units: (head pair)*2 + qt
    CCH = (IN_C + TIME_C) // P  # 6
    ECH = EXPAND_C // P  # 16

    # ---- long-lived pools ----
    const = tc.alloc_tile_pool(name="const", bufs=1)
    persist = tc.alloc_tile_pool(name="persist", bufs=1)

    ident = const.tile([P, P], dt_op)
    make_identity(nc, ident)
    eps_t = const.tile([P, 1], f32)
    nc.vector.memset(eps_t, EPS)
    s2i_t = const.tile([P, 1], f32)
    nc.vector.memset(s2i_t, 1.0 / F2_SCALE)

    # bias tiles (DMAs deferred into phase A so the x stream goes first)
    mb_row = const.tile([1, OUT_C], f32)
    mb_bc = const.tile([P, OUT_C], f32)
    f2b_row = const.tile([1, OUT_C], f32)
    f2b_bc = const.tile([P, OUT_C], f32)
    bqk_sb = const.tile([P, 8], f32)
    f1b_sb = const.tile([P, 16], f32)

    XT8 = persist.tile([P, KC // 2, 2, NTOK], dt8)  # fp8 copy
    qT = persist.tile([P, KC, NQ], dt_op)  # q^T (own rows), feature-major
    kT = persist.tile([P, KC, NTOK], dt_op)  # k^T, feature-major
    X2 = persist.tile([P, QCH, OUT_C], f32)  # x + attn + merge_b'
    tt_bc = const.tile([P, TIME_C], f32)  # t@time_w + time_b, bcast over rows
    st_tt = const.tile([P, 6], f32)  # bn_stats of tt_bc (same for all chunks)
    nrmA = persist.tile([P, QCH, IN_C + TIME_C], dt_op)  # ln2 normalized rows
    mvs = persist.tile([P, QCH, 2], f32)
    rstd8 = persist.tile([P, QCH], f32)

    def emit_ln2_stats(tci, phL):
        """LN2 stats for one token chunk (emit as soon as its X2 row is
        final so the chain overlaps the tail of attention)."""
        st2 = phL.tile([P, 3, 6], f32, tag="st2")
        nc.vector.bn_stats(out=st2[:, 0, :], in_=X2[:, tci, 0:256])
        nc.vector.bn_stats(out=st2[:, 1, :], in_=X2[:, tci, 256:512])
        nc.vector.tensor_copy(out=st2[:, 2, :], in_=st_tt)
        nc.vector.bn_aggr(out=mvs[:, tci, :], in_=st2)

    def emit_ln2(qt, phL, stats_done=False):
        """LN2 stats + normalize for one query tile's 4 token chunks."""
        if not stats_done:
            for tc4 in range(4):
                emit_ln2_stats(qt * 4 + tc4, phL)
        nc.scalar.activation(
            out=rstd8[:, qt * 4 : qt * 4 + 4],
            in_=mvs[:, qt * 4 : qt * 4 + 4, 1], func=AF.Sqrt, bias=eps_t,
        )
        nc.vector.reciprocal(
            out=rstd8[:, qt * 4 : qt * 4 + 4],
            in_=rstd8[:, qt * 4 : qt * 4 + 4],
        )
        for tc4 in range(4):
            tci = qt * 4 + tc4
            nc.vector.tensor_scalar(
                out=nrmA[:, tci, 0:IN_C], in0=X2[:, tci, :],
                scalar1=mvs[:, tci, 0:1], scalar2=rstd8[:, tci : tci + 1],
                op0=ALU.subtract, op1=ALU.mult,
            )
            nc.vector.tensor_scalar(
                out=nrmA[:, tci, IN_C:], in0=tt_bc,
                scalar1=mvs[:, tci, 0:1], scalar2=rstd8[:, tci : tci + 1],
                op0=ALU.subtract, op1=ALU.mult,
            )

    with (
        tc.tile_pool(name="phw", bufs=3) as phw,
        tc.tile_pool(name="phv", bufs=6) as phv,
        tc.tile_pool(name="phd", bufs=2) as phd,
        tc.tile_pool(name="phc", bufs=2) as phc,
    ):
        psV = tc.alloc_tile_pool(name="psV", bufs=2, space="PSUM")
        xtpool = tc.alloc_tile_pool(name="xtpool", bufs=1)
        XT = xtpool.tile([P, KC, NTOK], dt_op)  # ln1(x)^T, feature-major
        z_tiles = {}  # h -> fp8 DoubleRow-packed Z = ln1(x) @ WM_h
        wm_handles = {}

        def new_wm(h):
            wm_sb = phw.tile(
                [P, KC // 2, 2, IN_C], dt8, tag="wm", name=f"wm{h}"
            )
            nc.gpsimd.dma_start(out=wm_sb, in_=wm_d[:, h])
            return wm_sb

        def new_z(h):
            # 514 = 2*257 columns: [f0..f255, den, f256..f511, pad]. The den
            # column (value S_SCALE) makes the A@Z matmul emit the softmax
            # denominator as output column 256 of its first 257-wide chain.
            z_tiles[h] = phv.tile(
                [P, TCH // 2, 2, 514], dt8, tag="z", name=f"z{h}"
            )
            nc.vector.memset(z_tiles[h][:, :, :, 256:257], S_SCALE)

        def z_step(h, wm_sb, tci, evict=None):
            """Two DR matmuls + one copy eviction: Z chunk tci of head h."""
            pv = psV.tile([P, IN_C], f32, tag="pv")
            for kc2 in range(KC // 2):
                nc.tensor.matmul(
                    pv,
                    XT8[:, kc2, :, ts(tci, P)],
                    wm_sb[:, kc2, :, :],
                    start=(kc2 == 0),
                    stop=(kc2 == KC // 2 - 1),
                    perf_mode=mybir.MatmulPerfMode.DoubleRow,
                )
            # [2, 256] view skips the den column at 256
            dst = z_tiles[h][:, tci // 2, tci % 2, :].rearrange(
                "p (a c) -> p a c", a=2
            )[:, :, 0:256]
            src = pv.rearrange("p (a c) -> p a c", a=2)
            if evict is None:
                nc.vector.tensor_copy(out=dst, in_=src)
            else:
                evict.copy(out=dst, in_=src)

        # ---- phase A ----
        with (
            tc.tile_pool(name="pha", bufs=4) as pha,
            tc.tile_pool(name="pha1", bufs=4) as pha1,
            tc.tile_pool(name="psA", bufs=2, space="PSUM") as psA,
            tc.tile_pool(name="psAq", bufs=2, space="PSUM") as psAq,
            tc.tile_pool(name="psAt", bufs=1, space="PSUM") as psAt,
        ):
            # prefetch qkv weights + WM for head 0
            wqk_sb = pha.tile([P, KC, 2 * IN_C], dt_op, bufs=1)
            for c in range(KC):
                nc.gpsimd.dma_start(out=wqk_sb[:, c, :], in_=wqk_d[ts(c, P), :])
            wm_handles[0] = new_wm(0)
            # mb_bc must be written before the first X2-init add reads it
            nc.gpsimd.dma_start(out=mb_row, in_=mb_d[None, :])
            nc.gpsimd.partition_broadcast(mb_bc, mb_row)

            dma_engines = [nc.sync, nc.scalar]
            for tti in range(4):  # 512-token groups
                for sub in range(4):
                    tci = tti * 4 + sub
                    x_t = pha.tile([P, IN_C], dt_op)
                    dma_engines[tci % 2].dma_start(
                        out=x_t, in_=x_roll[ts(tci, P), :]
                    )
                    st = pha1.tile([P, 6], f32)
                    nc.vector.bn_stats(out=st, in_=x_t)
                    mv = pha1.tile([P, 2], f32)
                    nc.vector.bn_aggr(out=mv, in_=st)
                    rstd = pha1.tile([P, 1], f32)
                    nc.scalar.activation(
                        out=rstd, in_=mv[:, 1:2], func=AF.Sqrt, bias=eps_t
                    )
                    nc.vector.reciprocal(out=rstd, in_=rstd)
                    xn = pha.tile([P, IN_C], dt_op)
                    nc.vector.tensor_scalar(
                        out=xn, in0=x_t, scalar1=mv[:, 0:1], scalar2=rstd,
                        op0=ALU.subtract, op1=ALU.mult,
                    )
                    # PE transposes (DMA XBAR transpose measured slower:
                    # ~1.3us queue+engine occupancy per 512-col call)
                    for fc in range(KC):
                        pt = psA.tile([P, P], dt_op)
                        nc.tensor.transpose(pt, xn[:, ts(fc, P)], ident)
                        if fc % 2:
                            nc.scalar.copy(out=XT[:, fc, ts(tci, P)], in_=pt)
                        else:
                            nc.vector.tensor_copy(
                                out=XT[:, fc, ts(tci, P)], in_=pt
                            )
                    if tci < QCH:
                        # X2 residual init (x + merge_b') while x is
                        # resident, on the otherwise-idle gpsimd
                        nc.gpsimd.tensor_add(X2[:, tci, :], x_t, mb_bc)
                if tti == 0:
                    # deferred small DMAs, behind the first token group's
                    # x tiles but ahead of their consumers
                    nc.sync.dma_start(
                        out=bqk_sb, in_=bqk_d.rearrange("(c p) -> p c", p=P)
                    )
                    nc.scalar.dma_start(
                        out=f1b_sb, in_=f1b_d.rearrange("(c p) -> p c", p=P)
                    )
                    nc.gpsimd.dma_start(out=f2b_row, in_=f2b_d[None, :])
                    nc.gpsimd.partition_broadcast(f2b_bc, f2b_row)
                # q^T / k^T for this 512-token group. k chunks first
                # (and low feature-chunks first): head-pair 0's S matmuls
                # only need the fc=0 chunks, so they can start earliest.
                mcs = [4, 0, 5, 1, 6, 2, 7, 3] if tti < NQT else [4, 5, 6, 7]
                for mc in mcs:
                    pq = psAq.tile([P, 512], f32)
                    for kc in range(KC):
                        nc.tensor.matmul(
                            pq,
                            wqk_sb[:, kc, ts(mc, P)],
                            XT[:, kc, ts(tti, 512)],
                            start=(kc == 0),
                            stop=(kc == KC - 1),
                        )
                    dest = (
                        qT[:, mc, ts(tti, 512)]
                        if mc < 4
                        else kT[:, mc - 4, ts(tti, 512)]
                    )
                    nc.scalar.activation(
                        out=dest, in_=pq, func=AF.Identity,
                        bias=bqk_sb[:, mc : mc + 1],
                    )
                # fp8 copy of this token group's XT (one wide DVE cast)
                nc.vector.tensor_copy(
                    out=XT8[:, :, :, ts(tti, 512)].rearrange(
                        "p c j t -> p (c j) t"
                    ),
                    in_=XT[:, :, ts(tti, 512)],
                )
            # time embedding: tt = t @ time_w + time_b  -> broadcast tile
            tT = pha.tile([P, 2], f32, bufs=1)
            nc.gpsimd.dma_start(out=tT, in_=t_d.rearrange("(c p) -> p c", p=P))
            tw_sb = pha.tile([P, 2, TIME_C], f32, bufs=1)
            nc.gpsimd.dma_start(out=tw_sb, in_=tw_d.rearrange("(c p) m -> p c m", p=P))
            tb_sb = pha.tile([1, TIME_C], f32, bufs=1)
            nc.gpsimd.dma_start(out=tb_sb, in_=tb_d[None, :])
            ps_tt = psAt.tile([1, TIME_C], f32)
            for c in range(2):
                nc.tensor.matmul(
                    ps_tt, tT[:, c : c + 1], tw_sb[:, c, :],
                    start=(c == 0), stop=(c == 1),
                )
            tt_row = pha.tile([1, TIME_C], f32, bufs=1)
            nc.vector.tensor_add(tt_row, ps_tt, tb_sb)
            nc.gpsimd.partition_broadcast(tt_bc, tt_row)
            nc.vector.bn_stats(out=st_tt, in_=tt_bc)

        xtpool.release()

        # ---- phase C: software-pipelined attention units ----
        with (
            tc.tile_pool(name="psO", bufs=1, space="PSUM") as psO,
            tc.tile_pool(name="phs", bufs=3) as phs,
            tc.tile_pool(name="phsm", bufs=2) as phsm,
        ):
            psS = tc.alloc_tile_pool(name="psS", bufs=2, space="PSUM")
            # (X2 accumulator init happens in phase A while x is resident)

            units = {}  # u -> dict of tiles
            # Z production schedule: unit u produces head u's Z, one chunk
            # per key slot. Deadlines: az(v) during unit v+2 reads heads
            # (v&~1), (v&~1)+1 — both complete one unit earlier. Units 0/1
            # (no az yet) get the PE filler they need; phase A sheds its
            # old dense Z tail entirely.
            ZPROD = {u: [(u, list(range(16)))] for u in range(NU)}

            def az_groups(u, pool):
                """Closure emitting A@Z group g (0..7) of unit u: two DR
                accumulation chains (A-stationary, token-major out) of
                257/256 columns — chain A's column 256 is the softmax
                denominator (from Z's S_SCALE den column) — then one DVE
                reciprocal + scalar_tensor_tensor eviction into X2."""
                j, qt = u // 2, u % 2
                U = units[u]

                def emit_group(g):
                    head_b, tc4 = divmod(g, 4)
                    expS = U["exp"][:, head_b]
                    z_t = z_tiles[2 * j + head_b]
                    po = pool.tile([P, 2, 512], f32, tag="po")
                    for kc2 in range(TCH // 2):
                        nc.tensor.matmul(
                            po[:, 0, 0:257],
                            expS[:, kc2, :, ts(tc4, P)],
                            z_t[:, kc2, :, 0:257],
                            start=(kc2 == 0),
                            stop=(kc2 == TCH // 2 - 1),
                            perf_mode=mybir.MatmulPerfMode.DoubleRow,
                        )
                        nc.tensor.matmul(
                            po[:, 1, 0:256],
                            expS[:, kc2, :, ts(tc4, P)],
                            z_t[:, kc2, :, 257:513],
                            start=(kc2 == 0),
                            stop=(kc2 == TCH // 2 - 1),
                            perf_mode=mybir.MatmulPerfMode.DoubleRow,
                        )
                    tci = qt * 4 + tc4
                    rec = phsm.tile([P, 1], f32, tag="rec",
                                    name=f"rec{u}g{g}")
                    nc.vector.reciprocal(out=rec, in_=po[:, 0, 256:257])
                    x2v = X2[:, tci, :].rearrange("p (a c) -> p a c", a=2)
                    nc.vector.scalar_tensor_tensor(
                        out=x2v, in0=po[:, :, 0:256], scalar=rec,
                        in1=x2v, op0=ALU.mult, op1=ALU.add,
                    )

                return emit_group

            def emit_S(u):
                """S matmuls + exp for both heads of unit u (per key chunk,
                double-buffered PSUM), interleaved with A@Z of u-2 and Z
                production per the ZPROD schedule."""
                j, qt = u // 2, u % 2
                U = units[u] = {}
                expAB = U["exp"] = phs.tile(
                    [P, 2, TCH // 2, 2, 512], dt8, tag="exp", name=f"exp{u}"
                )
                zprod = []
                for h, chunks in ZPROD.get(u, ()):
                    if h not in wm_handles:
                        wm_handles[h] = new_wm(h)
                    new_z(h)
                    zprod.append((h, wm_handles[h], chunks))
                # prefetch the next unit's WM one unit (~18us) ahead
                for h2, _ in ZPROD.get(u + 1, ()):
                    if h2 not in wm_handles:
                        wm_handles[h2] = new_wm(h2)
                az = az_groups(u - 2, psO) if u >= 2 else None
                for kc in range(TCH):
                    kc2, jj = divmod(kc, 2)
                    pss = psS.tile([P, 2, 512], f32, tag="pss")
                    nc.tensor.matmul(
                        pss[:, 0, :], kT[0:64, j, ts(kc, P)],
                        qT[0:64, j, ts(qt, 512)],
                        start=True, stop=True,
                    )
                    nc.tensor.matmul(
                        pss[:, 1, :], kT[64:128, j, ts(kc, P)],
                        qT[64:128, j, ts(qt, 512)],
                        start=True, stop=True,
                    )
                    nc.scalar.activation(
                        out=expAB[:, :, kc2, jj, :], in_=pss, func=AF.Exp
                    )
                    for zi, (h, wm_h, chunks) in enumerate(zprod):
                        lo = len(chunks) * kc // TCH
                        hi = len(chunks) * (kc + 1) // TCH
                        for tci in chunks[lo:hi]:
                            z_step(h, wm_h, tci)
                    # A@Z on odd slots: the po eviction gets a full even
                    # slot of S/Z matmuls before the next chain needs the
                    # single-buffered po bank pair back
                    if az is not None and kc % 2 == 1:
                        az(kc // 2)

            for u in range(NU):
                emit_S(u)

            # flush: A@Z of units 6 and 7. psS and psV are done — release
            # their banks and run the flush double-buffered from psOf so the
            # back-to-back groups don't serialize on po evictions.
            psS.release()
            psOf = tc.alloc_tile_pool(name="psOf", bufs=2, space="PSUM")
            az = az_groups(NU - 2, psOf)
            for g in range(8):
                az(g)
                if g >= 4:
                    # X2 row (g-4) of qt0 is final after head B's add
                    emit_ln2_stats(g - 4, phd)
            emit_ln2(0, phd, stats_done=True)
            az = az_groups(NU - 1, psOf)
            for g in range(8):
                az(g)
                if g >= 4:
                    emit_ln2_stats(4 + g - 4, phd)
            psOf.release()

        psV.release()

        # ---- phase D: LN2 transposes + FFN + residual ----
        with (
            tc.tile_pool(name="phh", bufs=1) as phh,
            tc.tile_pool(name="phdw", bufs=1) as phdw,
            tc.tile_pool(name="psDt", bufs=3, space="PSUM") as psDt,
            tc.tile_pool(name="psDf", bufs=2, space="PSUM") as psDf,
        ):
            f1w_sb = phdw.tile([P, CCH // 2, 2, EXPAND_C], dt8)
            nc.gpsimd.dma_start(out=f1w_sb, in_=f1w_d)
            f2w_sb = phdw.tile([P, ECH // 2, 2, OUT_C], dt8)
            nc.gpsimd.dma_start(out=f2w_sb, in_=f2w_d)

            def emit_l2T(qt, act_only=False):
                # act_only: the DVE is busy with the attention flush when
                # qt0's transposes run, while the ACT queue is idle there
                l2T = phh.tile([P, CCH // 2, 2, 512], dt8, tag=f"l2T{qt}")
                for tc4 in range(4):
                    tci = qt * 4 + tc4
                    for fc in range(CCH):
                        pt2 = psDt.tile([P, P], dt_op)
                        nc.tensor.transpose(pt2, nrmA[:, tci, ts(fc, P)], ident)
                        dst = l2T[:, fc // 2, fc % 2, ts(tc4, P)]
                        if act_only or fc % 2:
                            nc.scalar.copy(out=dst, in_=pt2)
                        else:
                            nc.vector.tensor_copy(out=dst, in_=pt2)
                return l2T

            def emit_ff1(qt, l2T):
                # chunk pairs emitted with their accumulation steps
                # interleaved: adjacent matmuls hit different PSUM banks,
                # so the PE dual-issues them (same-bank chains serialize)
                hT = phh.tile([P, ECH // 2, 2, 512], dt8, tag="hT", bufs=2)
                for mc2 in range(ECH // 2):
                    pfA = psDf.tile([P, 512], f32, tag="pf")
                    pfB = psDf.tile([P, 512], f32, tag="pf")
                    for kc in range(CCH // 2):
                        for sub, pf in ((0, pfA), (1, pfB)):
                            nc.tensor.matmul(
                                pf,
                                f1w_sb[:, kc, :, ts(2 * mc2 + sub, P)],
                                l2T[:, kc, :, :],
                                start=(kc == 0), stop=(kc == CCH // 2 - 1),
                                perf_mode=mybir.MatmulPerfMode.DoubleRow,
                            )
                    for sub, pf in ((0, pfA), (1, pfB)):
                        mc = 2 * mc2 + sub
                        nc.scalar.activation(
                            out=hT[:, mc2, sub, :], in_=pf, func=AF.Gelu,
                            bias=f1b_sb[:, mc : mc + 1], scale=1.0 / F1_SCALE,
                        )
                return hT

            def emit_ff2(qt, hT):
                for tc4p in range(2):
                    pgA = psDf.tile([P, OUT_C], f32, tag="pf")
                    pgB = psDf.tile([P, OUT_C], f32, tag="pf")
                    for kc in range(ECH // 2):
                        for sub, pg in ((0, pgA), (1, pgB)):
                            nc.tensor.matmul(
                                pg,
                                hT[:, kc, :, ts(2 * tc4p + sub, P)],
                                f2w_sb[:, kc, :, :],
                                start=(kc == 0), stop=(kc == ECH // 2 - 1),
                                perf_mode=mybir.MatmulPerfMode.DoubleRow,
                            )
                    for sub, pg in ((0, pgA), (1, pgB)):
                        tc4 = 2 * tc4p + sub
                        tci = qt * 4 + tc4
                        xb = phd.tile([P, OUT_C], f32, tag="xb")
                        nc.vector.tensor_add(xb, X2[:, tci, :], f2b_bc)
                        outt = phd.tile([P, OUT_C], f32, tag="outt")
                        nc.vector.scalar_tensor_tensor(
                            out=outt, in0=pg, scalar=s2i_t, in1=xb,
                            op0=ALU.mult, op1=ALU.add,
                        )
                        nc.sync.dma_start(out=out_d[ts(tci, P), :], in_=outt)

            l2T0 = emit_l2T(0, act_only=True)
            emit_ln2(1, phd, stats_done=True)
            hT0 = emit_ff1(0, l2T0)
            l2T1 = emit_l2T(1)
            emit_ff2(0, hT0)
            hT1 = emit_ff1(1, l2T1)
            emit_ff2(1, hT1)

    persist.release()
    const.release()


def _dedup_ldweights(nc):
    """Remove InstLdweights whose weights AP matches the immediately
    preceding load on the PE queue (only InstMatmults in between): the
    weights are still resident in the array, so the reload is pure LW-port
    traffic. Any sync on the removed load is merged into the next PE
    instruction. Runs pre-finalize (multi-wait sync still legal)."""
    from concourse import mybir

    PE = mybir.EngineType.PE
    removed = 0
    for fn in nc.m.functions:
        for bb in fn.blocks:
            insts = bb.instructions
            last_key = None
            pending_sync = []
            to_remove = []
            for i in insts:
                if getattr(i, "engine", None) != PE:
                    continue
                nm = type(i).__name__
                if pending_sync:
                    si = i.sync_info
                    waits = list(si.on_wait) if si else []
                    ups = list(si.on_update) if si else []
                    for ps in pending_sync:
                        waits += list(ps.on_wait)
                        ups += list(ps.on_update)
                    i.sync_info = mybir.SyncInfo(on_wait=waits, on_update=ups)
                    pending_sync = []
                if nm == "InstLdweights":
                    key = (
                        str(i.ins[0]),
                        str(i.perf_mode),
                        str(i.is_transpose),
                    )
                    if key == last_key:
                        if i.sync_info is not None and (
                            i.sync_info.on_wait or i.sync_info.on_update
                        ):
                            pending_sync.append(i.sync_info)
                        to_remove.append(i)
                        removed += 1
                        continue
                    last_key = key
                elif nm != "InstMatmult":
                    last_key = None
            for i in to_remove:
                insts.remove(i)
    return removed


def build_program(quant=QUANT):
    import concourse.bass as bass
    import concourse.tile as tile
    from concourse import mybir, bacc

    f32 = mybir.dt.float32
    dt_op = mybir.dt.bfloat16
    dt_w = dt_op

    nc = bacc.Bacc(None, target_bir_lowering=False)
    tens = {}

    def inp(name, shape, dt):
        tens[name] = nc.dram_tensor(name, list(shape), dt, kind="ExternalInput")

    inp("x_roll", (NTOK, IN_C), dt_w)
    inp("wqk", (IN_C, 2 * IN_C), dt_w)
    inp("bqk", (2 * IN_C,), f32)
    inp("wm", (P, H, IN_C // (2 * P), 2, IN_C), mybir.dt.float8e4)
    inp("merge_b", (OUT_C,), f32)
    inp("t_vec", (TIME_C,), f32)
    inp("time_w", (TIME_C, TIME_C), f32)
    inp("time_b", (TIME_C,), f32)
    inp("ff1_w", (P, (IN_C + TIME_C) // (2 * P), 2, EXPAND_C),
        mybir.dt.float8e4)
    inp("ff1_b", (EXPAND_C,), f32)
    inp("ff2_w", (P, EXPAND_C // (2 * P), 2, OUT_C), mybir.dt.float8e4)
    inp("ff2_b", (OUT_C,), f32)
    tens["out"] = nc.dram_tensor("out", [NQ, OUT_C], f32, kind="ExternalOutput")

    with tile.TileContext(nc) as tc:
        _emit(nc, tc, tens, dt_op)
    # NOTE: deleting "redundant" InstLdweights (same weights as the prior
    # load) makes things WORSE: walrus re-materializes the load inside the
    # matmul, serializing load+mult (+73us measured). Keep explicit loads.
    nc.finalize()
    return nc


def make_in_maps(x, t, ln1_g, ln1_b, qkv_w, qkv_b, merge_w, merge_b, time_w,
                 time_b, ln2_g, ln2_b, ff1_w, ff1_b, ff2_w, ff2_b, quant=QUANT):
    import ml_dtypes

    f = np.float32
    npdt = ml_dtypes.bfloat16

    x = np.asarray(x, f)
    qkv_w = np.asarray(qkv_w, f)
    qkv_b = np.asarray(qkv_b, f)
    # fold ln1 affine into qkv
    qkv_wf = np.asarray(ln1_g, f)[:, None] * qkv_w
    qkv_bf = qkv_b + np.asarray(ln1_b, f) @ qkv_w
    scale = HEAD_C**-0.5
    qkv_wf[:, : H * HEAD_C] *= scale
    qkv_bf[: H * HEAD_C] *= scale
    wqk = qkv_wf[:, : 2 * H * HEAD_C]
    bqk = qkv_bf[: 2 * H * HEAD_C]
    wv = qkv_wf[:, 2 * H * HEAD_C :]
    bv = qkv_bf[2 * H * HEAD_C :]
    merge_w = np.asarray(merge_w, f)
    # fold v bias into merge_b (softmax rows sum to 1)
    merge_bf = np.asarray(merge_b, f) + bv @ merge_w
    # fold merge_w into wv per head (scaled into fp8's normal range; the
    # 1/S_SCALE rides on the softmax reciprocal via the sca_t matmul rhs)
    wm = np.empty((IN_C, H * IN_C), f)
    for h in range(H):
        wm[:, h * IN_C : (h + 1) * IN_C] = (
            wv[:, h * IN_C : (h + 1) * IN_C]
            @ merge_w[h * IN_C : (h + 1) * IN_C, :]
        ) * S_SCALE
    # pack to the device layout [p, h, c, j, m]: row index = c*256+j*128+p,
    # so each head's slab is one contiguous 2KB line per partition
    wm = np.ascontiguousarray(
        wm.reshape(2, 2, P, H, IN_C).transpose(2, 3, 0, 1, 4)
    )
    # fold ln2 affine into ff1; scale + pack for fp8 DoubleRow
    # (row index = cc2*256 + jj*128 + p -> [p, cc2, jj, m], contiguous DMA)
    ff1_wf = np.asarray(ln2_g, f)[:, None] * np.asarray(ff1_w, f)
    ff1_bf = np.asarray(ff1_b, f) + np.asarray(ln2_b, f) @ np.asarray(ff1_w, f)
    ff1_wf = np.ascontiguousarray(
        (ff1_wf * F1_SCALE).reshape(3, 2, P, EXPAND_C).transpose(2, 0, 1, 3)
    )

    shared = {
        "wqk": wqk.astype(npdt),
        "bqk": bqk.astype(f),
        "wm": wm.astype(ml_dtypes.float8_e4m3),
        "merge_b": merge_bf.astype(f),
        "time_w": np.asarray(time_w, f),
        "time_b": np.asarray(time_b, f),
        "ff1_w": ff1_wf.astype(ml_dtypes.float8_e4m3),
        "ff1_b": ff1_bf.astype(f),
        "ff2_w": np.ascontiguousarray(
            (np.asarray(ff2_w, f) * F2_SCALE)
            .reshape(8, 2, P, OUT_C).transpose(2, 0, 1, 3)
        ).astype(ml_dtypes.float8_e4m3),
        "ff2_b": np.asarray(ff2_b, f),
    }
    in_maps = []
    for c in range(NCORES):
        b, half = divmod(c, 2)
        xb = x[b]
        x_roll = np.concatenate([xb[half * NQ :], xb[: half * NQ]], axis=0)
        m = dict(shared)
        m["x_roll"] = np.ascontiguousarray(x_roll).astype(npdt)
        m["t_vec"] = np.asarray(t, f)[b]
        in_maps.append(m)
    return in_maps


_CACHE = {}


def kernel(**inputs):
    from concourse.bass_utils import run_bass_kernel_spmd

    if "nc" not in _CACHE:
        _CACHE["nc"] = build_program(QUANT)
    nc = _CACHE["nc"]
    in_maps = make_in_maps(**inputs, quant=QUANT)
    res = run_bass_kernel_spmd(nc, in_maps, core_ids=list(range(NCORES)))
    out = np.stack([res.results[c]["out"] for c in range(NCORES)], axis=0)
    return out.reshape(B, N, OUT_C)



# revision 46
# speedup vs baseline: 1.3029x; 1.3029x over previous
"""Trainium2 Bass kernel for nn_AttentionWithTime (B=4, N=2048, in_c=512,
head_c=64, H=8, expand_c=2048, time_c=256), 8-core SPMD.

Sharding: token-parallel. Core c handles batch b=c//2 and query rows
(c%2)*1024 .. +1024 of that batch. Each core computes K/V for its whole
batch (2x redundant) so no cross-core collectives are needed; the host
splits inputs and concatenates the 8 per-core [1024, 512] outputs.

Host-side folds (pure input math, done once in kernel()):
  - ln1 gamma/beta folded into qkv_w/qkv_b
  - attention scale (head_c^-0.5) folded into the q columns
  - v bias folded into merge_b (softmax rows sum to 1)
  - merge_w folded into wv per head: WM_h = wv_h @ merge_w_h * S_SCALE
    (fp8; the scale keeps the product in e4m3's normal range and rides
    on the softmax reciprocal). This removes the on-device merge matmul:
    out_h = (A_h @ Z_h) * recT with Z = ln1(x) @ WM.
  - ln2 gamma/beta folded into ff1_w/ff1_b

Device pipeline (per core):
  A: LN1 + bf16 transposes -> XT (bf16) + one wide fp8 cast -> XT8;
     q^T/k^T. (Z production lives entirely in the units: unit u makes
     head u's Z, one chunk per key slot.)
  C: 8 software-pipelined units (u = head-pair*2 + query-tile). Unit u's
     kc loop interleaves, per key chunk: the 2-head row-packed S matmuls
     (K=64 at row groups 0-63/64-127, concurrent in the PE, exp per kc
     from double-buffered PSUM), the A@Z matmuls of unit u-2
     (A-stationary fp8 DoubleRow, token-major out, as two 257/256-col
     accumulation chains; Z carries a constant S_SCALE column at 256 so
     chain A's column 256 IS the softmax denominator — no separate
     reduction), and Z production per the ZPROD schedule (front-loaded
     into the ACT-bound units 0/1). Each A@Z group evicts via one DVE
     reciprocal + scalar_tensor_tensor into X2.
  D: LN2 (time concat) + bf16 transposes + FFN, overlapping the flush
     of units 6/7. gpsimd only ever runs DMA issues + partition
     broadcasts (mixing op families forces ~7us microcode reloads).
"""

import numpy as np

B, N, IN_C, HEAD_C, EXPAND_C, OUT_C, TIME_C, H = 4, 2048, 512, 64, 2048, 512, 256, 8
EPS = 1e-5
NCORES = 8
NTOK = N  # tokens per batch handled per core (keys)
NQ = N // 2  # query rows owned per core
P = 128
S_SCALE = 16.0  # host scale on WM = wv@merge_w before fp8 cast
F1_SCALE = 256.0  # host scale on ff1_w before fp8 cast (undone in gelu scale)
F2_SCALE = 256.0  # host scale on ff2_w before fp8 cast (undone at eviction)

QUANT = "bf16"


def _emit(nc, tc, tens, dt_op):
    import concourse.bass as bass
    from concourse import mybir
    from concourse.masks import make_identity
    from concourse.bass import ts

    f32 = mybir.dt.float32
    f32r = mybir.dt.float32r
    dt8 = mybir.dt.float8e4
    AF = mybir.ActivationFunctionType
    ALU = mybir.AluOpType

    x_roll = tens["x_roll"][:]
    wqk_d = tens["wqk"][:]
    bqk_d = tens["bqk"][:]
    wm_d = tens["wm"][:]
    mb_d = tens["merge_b"][:]
    t_d = tens["t_vec"][:]
    tw_d = tens["time_w"][:]
    tb_d = tens["time_b"][:]
    f1w_d = tens["ff1_w"][:]
    f1b_d = tens["ff1_b"][:]
    f2w_d = tens["ff2_w"][:]
    f2b_d = tens["ff2_b"][:]
    out_d = tens["out"][:]

    KC = IN_C // P  # 4 feature chunks of x
    TCH = NTOK // P  # 16 token chunks per batch
    QCH = NQ // P  # 8 own-token chunks
    NQT = NQ // 512  # 2 query tiles of 512
    NU = H  # pipeline units: (head pair)*2 + qt
    CCH = (IN_C + TIME_C) // P  # 6
    ECH = EXPAND_C // P  # 16

    # ---- long-lived pools ----
    const = tc.alloc_tile_pool(name="const", bufs=1)
    persist = tc.alloc_tile_pool(name="persist", bufs=1)

    ident = const.tile([P, P], dt_op)
    make_identity(nc, ident)
    eps_t = const.tile([P, 1], f32)
    nc.vector.memset(eps_t, EPS)
    s2i_t = const.tile([P, 1], f32)
    nc.vector.memset(s2i_t, 1.0 / F2_SCALE)

    # bias tiles (DMAs deferred into phase A so the x stream goes first)
    mb_row = const.tile([1, OUT_C], f32)
    mb_bc = const.tile([P, OUT_C], f32)
    f2b_row = const.tile([1, OUT_C], f32)
    f2b_bc = const.tile([P, OUT_C], f32)
    bqk_sb = const.tile([P, 8], f32)
    f1b_sb = const.tile([P, 16], f32)

    XT8 = persist.tile([P, KC // 2, 2, NTOK], dt8)  # fp8 copy
    qT = persist.tile([P, KC, NQ], dt_op)  # q^T (own rows), feature-major
    kT = persist.tile([P, KC, NTOK], dt_op)  # k^T, feature-major
    X2 = persist.tile([P, QCH, OUT_C], f32)  # x + attn + merge_b'
    tt_bc = const.tile([P, TIME_C], f32)  # t@time_w + time_b, bcast over rows
    st_tt = const.tile([P, 6], f32)  # bn_stats of tt_bc (same for all chunks)
    nrmA = persist.tile([P, QCH, IN_C + TIME_C], dt_op)  # ln2 normalized rows
    mvs = persist.tile([P, QCH, 2], f32)
    rstd8 = persist.tile([P, QCH], f32)

    def emit_ln2_stats(tci, phL):
        """LN2 stats for one token chunk (emit as soon as its X2 row is
        final so the chain overlaps the tail of attention)."""
        st2 = phL.tile([P, 3, 6], f32, tag="st2")
        nc.vector.bn_stats(out=st2[:, 0, :], in_=X2[:, tci, 0:256])
        nc.vector.bn_stats(out=st2[:, 1, :], in_=X2[:, tci, 256:512])
        nc.vector.tensor_copy(out=st2[:, 2, :], in_=st_tt)
        nc.vector.bn_aggr(out=mvs[:, tci, :], in_=st2)

    def emit_ln2(qt, phL, stats_done=False):
        """LN2 stats + normalize for one query tile's 4 token chunks."""
        if not stats_done:
            for tc4 in range(4):
                emit_ln2_stats(qt * 4 + tc4, phL)
        nc.scalar.activation(
            out=rstd8[:, qt * 4 : qt * 4 + 4],
            in_=mvs[:, qt * 4 : qt * 4 + 4, 1], func=AF.Sqrt, bias=eps_t,
        )
        nc.vector.reciprocal(
            out=rstd8[:, qt * 4 : qt * 4 + 4],
            in_=rstd8[:, qt * 4 : qt * 4 + 4],
        )
        for tc4 in range(4):
            tci = qt * 4 + tc4
            nc.vector.tensor_scalar(
                out=nrmA[:, tci, 0:IN_C], in0=X2[:, tci, :],
                scalar1=mvs[:, tci, 0:1], scalar2=rstd8[:, tci : tci + 1],
                op0=ALU.subtract, op1=ALU.mult,
            )
            nc.vector.tensor_scalar(
                out=nrmA[:, tci, IN_C:], in0=tt_bc,
                scalar1=mvs[:, tci, 0:1], scalar2=rstd8[:, tci : tci + 1],
                op0=ALU.subtract, op1=ALU.mult,
            )

    with (
        tc.tile_pool(name="phw", bufs=3) as phw,
        tc.tile_pool(name="phv", bufs=6) as phv,
        tc.tile_pool(name="phd", bufs=2) as phd,
        tc.tile_pool(name="phc", bufs=2) as phc,
    ):
        psV = tc.alloc_tile_pool(name="psV", bufs=2, space="PSUM")
        xtpool = tc.alloc_tile_pool(name="xtpool", bufs=1)
        XT = xtpool.tile([P, KC, NTOK], dt_op)  # ln1(x)^T, feature-major
        z_tiles = {}  # h -> fp8 DoubleRow-packed Z = ln1(x) @ WM_h
        wm_handles = {}

        def new_wm(h):
            wm_sb = phw.tile(
                [P, KC // 2, 2, IN_C], dt8, tag="wm", name=f"wm{h}"
            )
            nc.gpsimd.dma_start(out=wm_sb, in_=wm_d[:, h])
            return wm_sb

        def new_z(h):
            # 514 = 2*257 columns: [f0..f255, den, f256..f511, pad]. The den
            # column (value S_SCALE) makes the A@Z matmul emit the softmax
            # denominator as output column 256 of its first 257-wide chain.
            z_tiles[h] = phv.tile(
                [P, TCH // 2, 2, 514], dt8, tag="z", name=f"z{h}"
            )
            nc.vector.memset(z_tiles[h][:, :, :, 256:257], S_SCALE)

        def z_step(h, wm_sb, tci, evict=None):
            """Two DR matmuls + one copy eviction: Z chunk tci of head h."""
            pv = psV.tile([P, IN_C], f32, tag="pv")
            for kc2 in range(KC // 2):
                nc.tensor.matmul(
                    pv,
                    XT8[:, kc2, :, ts(tci, P)],
                    wm_sb[:, kc2, :, :],
                    start=(kc2 == 0),
                    stop=(kc2 == KC // 2 - 1),
                    perf_mode=mybir.MatmulPerfMode.DoubleRow,
                )
            # [2, 256] view skips the den column at 256
            dst = z_tiles[h][:, tci // 2, tci % 2, :].rearrange(
                "p (a c) -> p a c", a=2
            )[:, :, 0:256]
            src = pv.rearrange("p (a c) -> p a c", a=2)
            if evict is None:
                nc.vector.tensor_copy(out=dst, in_=src)
            else:
                evict.copy(out=dst, in_=src)

        # ---- phase A ----
        with (
            tc.tile_pool(name="pha", bufs=4) as pha,
            tc.tile_pool(name="pha1", bufs=4) as pha1,
            tc.tile_pool(name="psA", bufs=2, space="PSUM") as psA,
            tc.tile_pool(name="psAq", bufs=2, space="PSUM") as psAq,
            tc.tile_pool(name="psAt", bufs=1, space="PSUM") as psAt,
        ):
            # prefetch qkv weights + WM for head 0
            wqk_sb = pha.tile([P, KC, 2 * IN_C], dt_op, bufs=1)
            for c in range(KC):
                nc.gpsimd.dma_start(out=wqk_sb[:, c, :], in_=wqk_d[ts(c, P), :])
            wm_handles[0] = new_wm(0)
            # mb_bc must be written before the first X2-init add reads it
            nc.gpsimd.dma_start(out=mb_row, in_=mb_d[None, :])
            nc.gpsimd.partition_broadcast(mb_bc, mb_row)

            dma_engines = [nc.sync, nc.scalar]
            for tti in range(4):  # 512-token groups
                for sub in range(4):
                    tci = tti * 4 + sub
                    x_t = pha.tile([P, IN_C], dt_op)
                    dma_engines[tci % 2].dma_start(
                        out=x_t, in_=x_roll[ts(tci, P), :]
                    )
                    st = pha1.tile([P, 6], f32)
                    nc.vector.bn_stats(out=st, in_=x_t)
                    mv = pha1.tile([P, 2], f32)
                    nc.vector.bn_aggr(out=mv, in_=st)
                    rstd = pha1.tile([P, 1], f32)
                    nc.scalar.activation(
                        out=rstd, in_=mv[:, 1:2], func=AF.Sqrt, bias=eps_t
                    )
                    nc.vector.reciprocal(out=rstd, in_=rstd)
                    xn = pha.tile([P, IN_C], dt_op)
                    nc.vector.tensor_scalar(
                        out=xn, in0=x_t, scalar1=mv[:, 0:1], scalar2=rstd,
                        op0=ALU.subtract, op1=ALU.mult,
                    )
                    # PE transposes (DMA XBAR transpose measured slower:
                    # ~1.3us queue+engine occupancy per 512-col call)
                    for fc in range(KC):
                        pt = psA.tile([P, P], dt_op)
                        nc.tensor.transpose(pt, xn[:, ts(fc, P)], ident)
                        if fc % 2:
                            nc.scalar.copy(out=XT[:, fc, ts(tci, P)], in_=pt)
                        else:
                            nc.vector.tensor_copy(
                                out=XT[:, fc, ts(tci, P)], in_=pt
                            )
                    if tci < QCH:
                        # X2 residual init (x + merge_b') while x is
                        # resident (gpsimd.tensor_add costs ~7us ucode
                        # reloads per op-family switch — measured +109us)
                        nc.vector.tensor_add(X2[:, tci, :], x_t, mb_bc)
                if tti == 0:
                    # deferred small DMAs, behind the first token group's
                    # x tiles but ahead of their consumers
                    nc.sync.dma_start(
                        out=bqk_sb, in_=bqk_d.rearrange("(c p) -> p c", p=P)
                    )
                    nc.scalar.dma_start(
                        out=f1b_sb, in_=f1b_d.rearrange("(c p) -> p c", p=P)
                    )
                    nc.gpsimd.dma_start(out=f2b_row, in_=f2b_d[None, :])
                    nc.gpsimd.partition_broadcast(f2b_bc, f2b_row)
                # q^T / k^T for this 512-token group. k chunks first
                # (and low feature-chunks first): head-pair 0's S matmuls
                # only need the fc=0 chunks, so they can start earliest.
                mcs = [4, 0, 5, 1, 6, 2, 7, 3] if tti < NQT else [4, 5, 6, 7]
                for mc in mcs:
                    pq = psAq.tile([P, 512], f32)
                    for kc in range(KC):
                        nc.tensor.matmul(
                            pq,
                            wqk_sb[:, kc, ts(mc, P)],
                            XT[:, kc, ts(tti, 512)],
                            start=(kc == 0),
                            stop=(kc == KC - 1),
                        )
                    dest = (
                        qT[:, mc, ts(tti, 512)]
                        if mc < 4
                        else kT[:, mc - 4, ts(tti, 512)]
                    )
                    nc.scalar.activation(
                        out=dest, in_=pq, func=AF.Identity,
                        bias=bqk_sb[:, mc : mc + 1],
                    )
                # fp8 copy of this token group's XT (one wide DVE cast)
                nc.vector.tensor_copy(
                    out=XT8[:, :, :, ts(tti, 512)].rearrange(
                        "p c j t -> p (c j) t"
                    ),
                    in_=XT[:, :, ts(tti, 512)],
                )
            # time embedding: tt = t @ time_w + time_b  -> broadcast tile
            tT = pha.tile([P, 2], f32, bufs=1)
            nc.gpsimd.dma_start(out=tT, in_=t_d.rearrange("(c p) -> p c", p=P))
            tw_sb = pha.tile([P, 2, TIME_C], f32, bufs=1)
            nc.gpsimd.dma_start(out=tw_sb, in_=tw_d.rearrange("(c p) m -> p c m", p=P))
            tb_sb = pha.tile([1, TIME_C], f32, bufs=1)
            nc.gpsimd.dma_start(out=tb_sb, in_=tb_d[None, :])
            ps_tt = psAt.tile([1, TIME_C], f32)
            for c in range(2):
                nc.tensor.matmul(
                    ps_tt, tT[:, c : c + 1], tw_sb[:, c, :],
                    start=(c == 0), stop=(c == 1),
                )
            tt_row = pha.tile([1, TIME_C], f32, bufs=1)
            nc.vector.tensor_add(tt_row, ps_tt, tb_sb)
            nc.gpsimd.partition_broadcast(tt_bc, tt_row)
            nc.vector.bn_stats(out=st_tt, in_=tt_bc)

        xtpool.release()

        # ---- phase C: software-pipelined attention units ----
        with (
            tc.tile_pool(name="psO", bufs=1, space="PSUM") as psO,
            tc.tile_pool(name="phs", bufs=3) as phs,
            tc.tile_pool(name="phsm", bufs=2) as phsm,
        ):
            psS = tc.alloc_tile_pool(name="psS", bufs=2, space="PSUM")
            # (X2 accumulator init happens in phase A while x is resident)

            units = {}  # u -> dict of tiles
            # Z production schedule: unit u produces head u's Z, one chunk
            # per key slot. Deadlines: az(v) during unit v+2 reads heads
            # (v&~1), (v&~1)+1 — both complete one unit earlier. Units 0/1
            # (no az yet) get the PE filler they need; phase A sheds its
            # old dense Z tail entirely.
            ZPROD = {u: [(u, list(range(16)))] for u in range(NU)}

            def az_groups(u, pool):
                """Closure emitting A@Z group g (0..7) of unit u: two DR
                accumulation chains (A-stationary, token-major out) of
                257/256 columns — chain A's column 256 is the softmax
                denominator (from Z's S_SCALE den column) — then one DVE
                reciprocal + scalar_tensor_tensor eviction into X2."""
                j, qt = u // 2, u % 2
                U = units[u]

                def emit_group(g):
                    head_b, tc4 = divmod(g, 4)
                    expS = U["exp"][:, head_b]
                    z_t = z_tiles[2 * j + head_b]
                    po = pool.tile([P, 2, 512], f32, tag="po")
                    for kc2 in range(TCH // 2):
                        nc.tensor.matmul(
                            po[:, 0, 0:257],
                            expS[:, kc2, :, ts(tc4, P)],
                            z_t[:, kc2, :, 0:257],
                            start=(kc2 == 0),
                            stop=(kc2 == TCH // 2 - 1),
                            perf_mode=mybir.MatmulPerfMode.DoubleRow,
                        )
                        nc.tensor.matmul(
                            po[:, 1, 0:256],
                            expS[:, kc2, :, ts(tc4, P)],
                            z_t[:, kc2, :, 257:513],
                            start=(kc2 == 0),
                            stop=(kc2 == TCH // 2 - 1),
                            perf_mode=mybir.MatmulPerfMode.DoubleRow,
                        )
                    tci = qt * 4 + tc4
                    rec = phsm.tile([P, 1], f32, tag="rec",
                                    name=f"rec{u}g{g}")
                    nc.vector.reciprocal(out=rec, in_=po[:, 0, 256:257])
                    x2v = X2[:, tci, :].rearrange("p (a c) -> p a c", a=2)
                    nc.vector.scalar_tensor_tensor(
                        out=x2v, in0=po[:, :, 0:256], scalar=rec,
                        in1=x2v, op0=ALU.mult, op1=ALU.add,
                    )

                return emit_group

            def emit_S(u):
                """S matmuls + exp for both heads of unit u (per key chunk,
                double-buffered PSUM), interleaved with A@Z of u-2 and Z
                production per the ZPROD schedule."""
                j, qt = u // 2, u % 2
                U = units[u] = {}
                expAB = U["exp"] = phs.tile(
                    [P, 2, TCH // 2, 2, 512], dt8, tag="exp", name=f"exp{u}"
                )
                zprod = []
                for h, chunks in ZPROD.get(u, ()):
                    if h not in wm_handles:
                        wm_handles[h] = new_wm(h)
                    new_z(h)
                    zprod.append((h, wm_handles[h], chunks))
                # prefetch the next unit's WM one unit (~18us) ahead
                for h2, _ in ZPROD.get(u + 1, ()):
                    if h2 not in wm_handles:
                        wm_handles[h2] = new_wm(h2)
                az = az_groups(u - 2, psO) if u >= 2 else None
                for kc in range(TCH):
                    kc2, jj = divmod(kc, 2)
                    pss = psS.tile([P, 2, 512], f32, tag="pss")
                    nc.tensor.matmul(
                        pss[:, 0, :], kT[0:64, j, ts(kc, P)],
                        qT[0:64, j, ts(qt, 512)],
                        start=True, stop=True,
                    )
                    nc.tensor.matmul(
                        pss[:, 1, :], kT[64:128, j, ts(kc, P)],
                        qT[64:128, j, ts(qt, 512)],
                        start=True, stop=True,
                    )
                    nc.scalar.activation(
                        out=expAB[:, :, kc2, jj, :], in_=pss, func=AF.Exp
                    )
                    for zi, (h, wm_h, chunks) in enumerate(zprod):
                        lo = len(chunks) * kc // TCH
                        hi = len(chunks) * (kc + 1) // TCH
                        for tci in chunks[lo:hi]:
                            z_step(h, wm_h, tci)
                    # A@Z on odd slots: the po eviction gets a full even
                    # slot of S/Z matmuls before the next chain needs the
                    # single-buffered po bank pair back
                    if az is not None and kc % 2 == 1:
                        az(kc // 2)

            for u in range(NU):
                emit_S(u)

            # flush: A@Z of units 6 and 7. psS and psV are done — release
            # their banks and run the flush double-buffered from psOf so the
            # back-to-back groups don't serialize on po evictions.
            psS.release()
            psOf = tc.alloc_tile_pool(name="psOf", bufs=2, space="PSUM")
            az = az_groups(NU - 2, psOf)
            for g in range(8):
                az(g)
                if g >= 4:
                    # X2 row (g-4) of qt0 is final after head B's add
                    emit_ln2_stats(g - 4, phd)
            emit_ln2(0, phd, stats_done=True)
            az = az_groups(NU - 1, psOf)
            for g in range(8):
                az(g)
                if g >= 4:
                    emit_ln2_stats(4 + g - 4, phd)
            psOf.release()

        psV.release()

        # ---- phase D: LN2 transposes + FFN + residual ----
        with (
            tc.tile_pool(name="phh", bufs=1) as phh,
            tc.tile_pool(name="phdw", bufs=1) as phdw,
            tc.tile_pool(name="psDt", bufs=3, space="PSUM") as psDt,
            tc.tile_pool(name="psDf", bufs=2, space="PSUM") as psDf,
        ):
            f1w_sb = phdw.tile([P, CCH // 2, 2, EXPAND_C], dt8)
            nc.gpsimd.dma_start(out=f1w_sb, in_=f1w_d)
            f2w_sb = phdw.tile([P, ECH // 2, 2, OUT_C], dt8)
            nc.gpsimd.dma_start(out=f2w_sb, in_=f2w_d)

            def emit_l2T(qt, act_only=False):
                # act_only: the DVE is busy with the attention flush when
                # qt0's transposes run, while the ACT queue is idle there
                l2T = phh.tile([P, CCH // 2, 2, 512], dt8, tag=f"l2T{qt}")
                for tc4 in range(4):
                    tci = qt * 4 + tc4
                    for fc in range(CCH):
                        pt2 = psDt.tile([P, P], dt_op)
                        nc.tensor.transpose(pt2, nrmA[:, tci, ts(fc, P)], ident)
                        dst = l2T[:, fc // 2, fc % 2, ts(tc4, P)]
                        if act_only or fc % 2:
                            nc.scalar.copy(out=dst, in_=pt2)
                        else:
                            nc.vector.tensor_copy(out=dst, in_=pt2)
                return l2T

            def emit_ff1(qt, l2T):
                # chunk pairs emitted with their accumulation steps
                # interleaved: adjacent matmuls hit different PSUM banks,
                # so the PE dual-issues them (same-bank chains serialize)
                hT = phh.tile([P, ECH // 2, 2, 512], dt8, tag="hT", bufs=2)
                for mc2 in range(ECH // 2):
                    pfA = psDf.tile([P, 512], f32, tag="pf")
                    pfB = psDf.tile([P, 512], f32, tag="pf")
                    for kc in range(CCH // 2):
                        for sub, pf in ((0, pfA), (1, pfB)):
                            nc.tensor.matmul(
                                pf,
                                f1w_sb[:, kc, :, ts(2 * mc2 + sub, P)],
                                l2T[:, kc, :, :],
                                start=(kc == 0), stop=(kc == CCH // 2 - 1),
                                perf_mode=mybir.MatmulPerfMode.DoubleRow,
                            )
                    for sub, pf in ((0, pfA), (1, pfB)):
                        mc = 2 * mc2 + sub
                        nc.scalar.activation(
                            out=hT[:, mc2, sub, :], in_=pf, func=AF.Gelu,
                            bias=f1b_sb[:, mc : mc + 1], scale=1.0 / F1_SCALE,
                        )
                return hT

            def emit_ff2(qt, hT):
                for tc4p in range(2):
                    pgA = psDf.tile([P, OUT_C], f32, tag="pf")
                    pgB = psDf.tile([P, OUT_C], f32, tag="pf")
                    for kc in range(ECH // 2):
                        for sub, pg in ((0, pgA), (1, pgB)):
                            nc.tensor.matmul(
                                pg,
                                hT[:, kc, :, ts(2 * tc4p + sub, P)],
                                f2w_sb[:, kc, :, :],
                                start=(kc == 0), stop=(kc == ECH // 2 - 1),
                                perf_mode=mybir.MatmulPerfMode.DoubleRow,
                            )
                    for sub, pg in ((0, pgA), (1, pgB)):
                        tc4 = 2 * tc4p + sub
                        tci = qt * 4 + tc4
                        xb = phd.tile([P, OUT_C], f32, tag="xb")
                        nc.vector.tensor_add(xb, X2[:, tci, :], f2b_bc)
                        outt = phd.tile([P, OUT_C], f32, tag="outt")
                        nc.vector.scalar_tensor_tensor(
                            out=outt, in0=pg, scalar=s2i_t, in1=xb,
                            op0=ALU.mult, op1=ALU.add,
                        )
                        nc.sync.dma_start(out=out_d[ts(tci, P), :], in_=outt)

            l2T0 = emit_l2T(0, act_only=True)
            emit_ln2(1, phd, stats_done=True)
            hT0 = emit_ff1(0, l2T0)
            l2T1 = emit_l2T(1)
            emit_ff2(0, hT0)
            hT1 = emit_ff1(1, l2T1)
            emit_ff2(1, hT1)

    persist.release()
    const.release()


def _dedup_ldweights(nc):
    """Remove InstLdweights whose weights AP matches the immediately
    preceding load on the PE queue (only InstMatmults in between): the
    weights are still resident in the array, so the reload is pure LW-port
    traffic. Any sync on the removed load is merged into the next PE
    instruction. Runs pre-finalize (multi-wait sync still legal)."""
    from concourse import mybir

    PE = mybir.EngineType.PE
    removed = 0
    for fn in nc.m.functions:
        for bb in fn.blocks:
            insts = bb.instructions
            last_key = None
            pending_sync = []
            to_remove = []
            for i in insts:
                if getattr(i, "engine", None) != PE:
                    continue
                nm = type(i).__name__
                if pending_sync:
                    si = i.sync_info
                    waits = list(si.on_wait) if si else []
                    ups = list(si.on_update) if si else []
                    for ps in pending_sync:
                        waits += list(ps.on_wait)
                        ups += list(ps.on_update)
                    i.sync_info = mybir.SyncInfo(on_wait=waits, on_update=ups)
                    pending_sync = []
                if nm == "InstLdweights":
                    key = (
                        str(i.ins[0]),
                        str(i.perf_mode),
                        str(i.is_transpose),
                    )
                    if key == last_key:
                        if i.sync_info is not None and (
                            i.sync_info.on_wait or i.sync_info.on_update
                        ):
                            pending_sync.append(i.sync_info)
                        to_remove.append(i)
                        removed += 1
                        continue
                    last_key = key
                elif nm != "InstMatmult":
                    last_key = None
            for i in to_remove:
                insts.remove(i)
    return removed


def build_program(quant=QUANT):
    import concourse.bass as bass
    import concourse.tile as tile
    from concourse import mybir, bacc

    f32 = mybir.dt.float32
    dt_op = mybir.dt.bfloat16
    dt_w = dt_op

    nc = bacc.Bacc(None, target_bir_lowering=False)
    tens = {}

    def inp(name, shape, dt):
        tens[name] = nc.dram_tensor(name, list(shape), dt, kind="ExternalInput")

    inp("x_roll", (NTOK, IN_C), dt_w)
    inp("wqk", (IN_C, 2 * IN_C), dt_w)
    inp("bqk", (2 * IN_C,), f32)
    inp("wm", (P, H, IN_C // (2 * P), 2, IN_C), mybir.dt.float8e4)
    inp("merge_b", (OUT_C,), f32)
    inp("t_vec", (TIME_C,), f32)
    inp("time_w", (TIME_C, TIME_C), f32)
    inp("time_b", (TIME_C,), f32)
    inp("ff1_w", (P, (IN_C + TIME_C) // (2 * P), 2, EXPAND_C),
        mybir.dt.float8e4)
    inp("ff1_b", (EXPAND_C,), f32)
    inp("ff2_w", (P, EXPAND_C // (2 * P), 2, OUT_C), mybir.dt.float8e4)
    inp("ff2_b", (OUT_C,), f32)
    tens["out"] = nc.dram_tensor("out", [NQ, OUT_C], f32, kind="ExternalOutput")

    with tile.TileContext(nc) as tc:
        _emit(nc, tc, tens, dt_op)
    # NOTE: deleting "redundant" InstLdweights (same weights as the prior
    # load) makes things WORSE: walrus re-materializes the load inside the
    # matmul, serializing load+mult (+73us measured). Keep explicit loads.
    nc.finalize()
    return nc


def make_in_maps(x, t, ln1_g, ln1_b, qkv_w, qkv_b, merge_w, merge_b, time_w,
                 time_b, ln2_g, ln2_b, ff1_w, ff1_b, ff2_w, ff2_b, quant=QUANT):
    import ml_dtypes

    f = np.float32
    npdt = ml_dtypes.bfloat16

    x = np.asarray(x, f)
    qkv_w = np.asarray(qkv_w, f)
    qkv_b = np.asarray(qkv_b, f)
    # fold ln1 affine into qkv
    qkv_wf = np.asarray(ln1_g, f)[:, None] * qkv_w
    qkv_bf = qkv_b + np.asarray(ln1_b, f) @ qkv_w
    scale = HEAD_C**-0.5
    qkv_wf[:, : H * HEAD_C] *= scale
    qkv_bf[: H * HEAD_C] *= scale
    wqk = qkv_wf[:, : 2 * H * HEAD_C]
    bqk = qkv_bf[: 2 * H * HEAD_C]
    wv = qkv_wf[:, 2 * H * HEAD_C :]
    bv = qkv_bf[2 * H * HEAD_C :]
    merge_w = np.asarray(merge_w, f)
    # fold v bias into merge_b (softmax rows sum to 1)
    merge_bf = np.asarray(merge_b, f) + bv @ merge_w
    # fold merge_w into wv per head (scaled into fp8's normal range; the
    # 1/S_SCALE rides on the softmax reciprocal via the sca_t matmul rhs)
    wm = np.empty((IN_C, H * IN_C), f)
    for h in range(H):
        wm[:, h * IN_C : (h + 1) * IN_C] = (
            wv[:, h * IN_C : (h + 1) * IN_C]
            @ merge_w[h * IN_C : (h + 1) * IN_C, :]
        ) * S_SCALE
    # pack to the device layout [p, h, c, j, m]: row index = c*256+j*128+p,
    # so each head's slab is one contiguous 2KB line per partition
    wm = np.ascontiguousarray(
        wm.reshape(2, 2, P, H, IN_C).transpose(2, 3, 0, 1, 4)
    )
    # fold ln2 affine into ff1; scale + pack for fp8 DoubleRow
    # (row index = cc2*256 + jj*128 + p -> [p, cc2, jj, m], contiguous DMA)
    ff1_wf = np.asarray(ln2_g, f)[:, None] * np.asarray(ff1_w, f)
    ff1_bf = np.asarray(ff1_b, f) + np.asarray(ln2_b, f) @ np.asarray(ff1_w, f)
    ff1_wf = np.ascontiguousarray(
        (ff1_wf * F1_SCALE).reshape(3, 2, P, EXPAND_C).transpose(2, 0, 1, 3)
    )

    shared = {
        "wqk": wqk.astype(npdt),
        "bqk": bqk.astype(f),
        "wm": wm.astype(ml_dtypes.float8_e4m3),
        "merge_b": merge_bf.astype(f),
        "time_w": np.asarray(time_w, f),
        "time_b": np.asarray(time_b, f),
        "ff1_w": ff1_wf.astype(ml_dtypes.float8_e4m3),
        "ff1_b": ff1_bf.astype(f),
        "ff2_w": np.ascontiguousarray(
            (np.asarray(ff2_w, f) * F2_SCALE)
            .reshape(8, 2, P, OUT_C).transpose(2, 0, 1, 3)
        ).astype(ml_dtypes.float8_e4m3),
        "ff2_b": np.asarray(ff2_b, f),
    }
    in_maps = []
    for c in range(NCORES):
        b, half = divmod(c, 2)
        xb = x[b]
        x_roll = np.concatenate([xb[half * NQ :], xb[: half * NQ]], axis=0)
        m = dict(shared)
        m["x_roll"] = np.ascontiguousarray(x_roll).astype(npdt)
        m["t_vec"] = np.asarray(t, f)[b]
        in_maps.append(m)
    return in_maps


_CACHE = {}


def kernel(**inputs):
    from concourse.bass_utils import run_bass_kernel_spmd

    if "nc" not in _CACHE:
        _CACHE["nc"] = build_program(QUANT)
    nc = _CACHE["nc"]
    in_maps = make_in_maps(**inputs, quant=QUANT)
    res = run_bass_kernel_spmd(nc, in_maps, core_ids=list(range(NCORES)))
    out = np.stack([res.results[c]["out"] for c in range(NCORES)], axis=0)
    return out.reshape(B, N, OUT_C)

